# revision 1
# baseline (speedup 1.0000x reference)
"""Trainium2 Bass kernel for nn_Actor GNN message passing (8 NeuronCores).

Model (per reference): T=14 steps over N=2048 nodes. Per step:
  SAGE aggregation over dense 0/1 adjacency -> L2 normalize -> relu (xi),
  delayed-state mixing merged_k = A_norm @ delayed[1+k], 3-step map-LSTM,
  then a 12-step temporal LSTM over [mapped, obs, gamma] and a linear head.

Distribution: nodes sharded 256/core. The recurrence unrolls:
  xi_t        : local (column-block of A, no collective)
  merged1_t   = A_norm_t @ xi_{t-1}      (needs all-gathered xi)
  merged0_t   = A_norm_t @ merged1_{t-1} (needs all-gathered merged1)
so cross-core traffic is two batched AllGathers (bf16). The 1/rowsum
normalization is applied on the producer side (each shard scales its own
rows by the next step's local inverse rowsum before the gather), so the
consumer matmuls use gathered tiles directly. A travels as fp8e4m3 (0/1
exact); matmuls mix fp8/bf16 (verified exact on HW); LSTMs run in
float32r at full PE speed.
"""
import sys

sys.path.insert(0, "/opt/trn_rl_repo")

import numpy as np
import ml_dtypes

from concourse import bacc, mybir, tile
from concourse import bass2jax

F32 = mybir.dt.float32
F32R = mybir.dt.float32r
BF16 = mybir.dt.bfloat16
FP8 = mybir.dt.float8e4
AF = mybir.ActivationFunctionType

N, T, KSEQ, LEN = 2048, 14, 3, 12
H, IN_DIM, OBS_DIM, GAMMA_DIM = 128, 6, 6, 2
D = H + OBS_DIM + GAMMA_DIM  # 136
NCORES = 8
P = N // NCORES    # 256 nodes per core
NT = N // 128      # 16 m-tiles of 128
B_ALL = T * P      # 3584 map-LSTM batch
CH = 512           # LSTM chunk width
NCH = B_ALL // CH  # 7 chunks

_cached = {}


def build():
    nc = bacc.Bacc("TRN2", target_bir_lowering=False, debug=False,
                   num_devices=NCORES)

    # ---- DRAM parameters ----
    tblk = nc.dram_tensor("tblk", [T - 1, N, P], FP8, kind="ExternalInput")
    cblk = nc.dram_tensor("cblk", [T, N, P], FP8, kind="ExternalInput")
    xext = nc.dram_tensor("xext", [T, N, 8], BF16, kind="ExternalInput")
    xT = nc.dram_tensor("xT", [T, 8, P], BF16, kind="ExternalInput")
    obsgamT = nc.dram_tensor("obsgamT", [T, 8, P], F32, kind="ExternalInput")
    wlT = nc.dram_tensor("wlT", [8, H], BF16, kind="ExternalInput")
    wrT = nc.dram_tensor("wrT", [8, H], BF16, kind="ExternalInput")
    blrow = nc.dram_tensor("blrow", [1, H], BF16, kind="ExternalInput")
    identin = nc.dram_tensor("identin", [128, 128], BF16, kind="ExternalInput")
    mwihT = nc.dram_tensor("mwihT", [H, 4 * H], F32, kind="ExternalInput")
    mwhhT = nc.dram_tensor("mwhhT", [H, 4 * H], F32, kind="ExternalInput")
    mbias = nc.dram_tensor("mbias", [H, 4], F32, kind="ExternalInput")
    # temporal weights: cols 0..511 main gates; 512..639 tail gates padded
    # to 32-partition offsets (gate g tail at cols 512+32g .. 512+32g+8)
    lwihT_top = nc.dram_tensor("lwihT_top", [H, 640], F32, kind="ExternalInput")
    lwihT_tail = nc.dram_tensor("lwihT_tail", [8, 640], F32, kind="ExternalInput")
    lwhhT_top = nc.dram_tensor("lwhhT_top", [H, 640], F32, kind="ExternalInput")
    lwhhT_tail = nc.dram_tensor("lwhhT_tail", [8, 640], F32, kind="ExternalInput")
    lb_main = nc.dram_tensor("lb_main", [H, 4], F32, kind="ExternalInput")
    lb_tail = nc.dram_tensor("lb_tail", [128, 1], F32, kind="ExternalInput")
    linwT_top = nc.dram_tensor("linwT_top", [H, D], F32, kind="ExternalInput")
    linwT_tail = nc.dram_tensor("linwT_tail", [8, D], F32, kind="ExternalInput")
    linb_main = nc.dram_tensor("linb_main", [H, 1], F32, kind="ExternalInput")
    linb_tail = nc.dram_tensor("linb_tail", [8, 1], F32, kind="ExternalInput")
    lin1wT_top = nc.dram_tensor("lin1wT_top", [H, 2], F32, kind="ExternalInput")
    lin1wT_tail = nc.dram_tensor("lin1wT_tail", [8, 2], F32, kind="ExternalInput")
    lin1b = nc.dram_tensor("lin1b", [2, 1], F32, kind="ExternalInput")

    out_ext = nc.dram_tensor("out", [2, P], F32, kind="ExternalOutput")

    # ---- internal DRAM collective buffers (chunked for overlap) ----
    # xi chunk A: t=0..5 (6 steps), chunk B: t=6..12 (7 steps)
    # m1 chunk A: t=1..6 (6 steps), chunk B: t=7..12 (6 steps)
    xi_sh = nc.dram_tensor("xi_sh", [13 * P, H], BF16)
    xi_gA = nc.dram_tensor("xi_gA", [NCORES * 6 * P, H], BF16, addr_space="Shared")
    xi_gB = nc.dram_tensor("xi_gB", [NCORES * 7 * P, H], BF16, addr_space="Shared")
    m1_sh = nc.dram_tensor("m1_sh", [12 * P, H], BF16)
    m1_gA = nc.dram_tensor("m1_gA", [NCORES * 6 * P, H], BF16, addr_space="Shared")
    m1_gB = nc.dram_tensor("m1_gB", [NCORES * 6 * P, H], BF16, addr_space="Shared")

    RG = [list(range(NCORES))]

    with tile.TileContext(nc) as tc:
        with (
            tc.tile_pool(name="wpool", bufs=1) as wpool,
            tc.tile_pool(name="big", bufs=1) as big,
            tc.tile_pool(name="sb", bufs=2) as sb,
            tc.tile_pool(name="sb3", bufs=3) as sb3,
            tc.tile_pool(name="sb1", bufs=1) as sb1,
            tc.tile_pool(name="ps", bufs=1, space="PSUM") as ps,
            tc.tile_pool(name="ps2", bufs=2, space="PSUM") as ps2,
        ):
            # ---------- weights ----------
            ones8 = wpool.tile([128, 1], FP8, tag="ones8")
            nc.vector.memset(ones8[:, :], 1.0)
            ones1 = wpool.tile([1, H], BF16, tag="ones1")
            nc.vector.memset(ones1[:, :], 1.0)
            onesf = wpool.tile([1, 1], F32, tag="onesf")
            nc.vector.memset(onesf[:, :], 1.0)
            ident = wpool.tile([128, 128], BF16, tag="ident")
            nc.sync.dma_start(out=ident[:, :], in_=identin[:, :])

            wlT_sb = wpool.tile([8, H], BF16, tag="wlT")
            nc.sync.dma_start(out=wlT_sb[:, :], in_=wlT[:, :])
            wrT_sb = wpool.tile([8, H], BF16, tag="wrT")
            nc.sync.dma_start(out=wrT_sb[:, :], in_=wrT[:, :])
            bl_sb = wpool.tile([1, H], BF16, tag="bl")
            nc.sync.dma_start(out=bl_sb[:, :], in_=blrow[:, :])

            def f32r_weight(name, src, shape):
                t_f32 = sb1.tile(shape, F32, tag="wld")
                nc.sync.dma_start(out=t_f32[:, :], in_=src[:, :])
                t_r = wpool.tile(shape, F32R, tag=name)
                nc.scalar.copy(t_r[:, :], t_f32[:, :])
                return t_r

            mwihT_r = f32r_weight("mwihT", mwihT, [H, 4 * H])
            mwihT_bf = wpool.tile([H, 4 * H], BF16, tag="mwihT_bf")
            nc.scalar.copy(mwihT_bf[:, :], mwihT_r[:, :])
            mwhhT_r = f32r_weight("mwhhT", mwhhT, [H, 4 * H])
            lwihT_top_r = f32r_weight("lwihT_top", lwihT_top, [H, 640])
            lwihT_tail_r = f32r_weight("lwihT_tail", lwihT_tail, [8, 640])
            lwhhT_top_r = f32r_weight("lwhhT_top", lwhhT_top, [H, 640])
            lwhhT_tail_r = f32r_weight("lwhhT_tail", lwhhT_tail, [8, 640])
            linwT_top_r = f32r_weight("linwT_top", linwT_top, [H, D])
            linwT_tail_r = f32r_weight("linwT_tail", linwT_tail, [8, D])
            lin1wT_top_r = f32r_weight("lin1wT_top", lin1wT_top, [H, 2])
            lin1wT_tail_r = f32r_weight("lin1wT_tail", lin1wT_tail, [8, 2])

            def f32_const(name, src, shape):
                t_ = wpool.tile(shape, F32, tag=name)
                nc.sync.dma_start(out=t_[:, :], in_=src[:, :])
                return t_

            mb_sb = f32_const("mb", mbias, [H, 4])
            lbm_sb = f32_const("lbm", lb_main, [H, 4])
            lbt_sb = f32_const("lbt", lb_tail, [128, 1])
            linbm_sb = f32_const("linbm", linb_main, [H, 1])
            linbt_sb = f32_const("linbt", linb_tail, [8, 1])
            lin1b_sb = f32_const("lin1b", lin1b, [2, 1])

            # persistent big arrays
            tres = big.tile([128, 13 * NT * P], FP8, tag="tres")  # t=1..13
            m0T = big.tile([128, B_ALL], BF16, tag="m0T")
            m1T = big.tile([128, B_ALL], BF16, tag="m1T")
            xiT = big.tile([128, B_ALL], BF16, tag="xiT")
            hT = big.tile([128, B_ALL], F32R, tag="hT")
            cT = big.tile([128, B_ALL], F32, tag="cT")
            # local inverse rowsums of step t+1, own node shard, node on
            # partitions: column t*2+mj
            invrowL = big.tile([128, 2 * 13], F32, tag="invrowL")

            nc.vector.memset(m1T[:, 0:P], 0.0)
            nc.vector.memset(m0T[:, 0:2 * P], 0.0)

            # ---------- phase 1: xi_t, local invrow_{t+1} ----------
            for t in range(T):
                cb = sb3.tile([128, NT * P], FP8, tag="cb")
                nc.sync.dma_start(
                    out=cb[:, :].rearrange("p (k f) -> p k f", f=P),
                    in_=cblk[t].rearrange("(k p) f -> p k f", p=128))
                xe = sb.tile([128, NT * 8], BF16, tag="xe")
                nc.sync.dma_start(
                    out=xe[:, :].rearrange("p (k s) -> p k s", s=8),
                    in_=xext[t].rearrange("(k p) s -> p k s", p=128))
                xT_sb = sb.tile([8, P], BF16, tag="xTs")
                nc.sync.dma_start(out=xT_sb[:, :], in_=xT[t])

                # tblk for step t+1: load into residency; local rowsum_{t+1}
                if t < T - 1:
                    base = t * NT * P
                    nc.sync.dma_start(
                        out=tres[:, base:base + NT * P]
                        .rearrange("p (k f) -> p k f", f=P),
                        in_=tblk[t].rearrange("(k p) f -> p k f", p=128))
                    rs_ps = ps.tile([1, P], F32, tag="rs")
                    for kt in range(NT):
                        nc.tensor.matmul(
                            rs_ps[:, :], ones8[:, :],
                            tres[:, base + kt * P: base + (kt + 1) * P],
                            start=(kt == 0), stop=(kt == NT - 1))
                    rs_row = sb.tile([1, P], F32, tag="rsrow")
                    nc.vector.reciprocal(rs_row[:, :], rs_ps[:, :])
                    for mj in range(2):
                        irT = ps.tile([128, 1], F32, tag="tr")
                        nc.tensor.transpose(
                            irT[:, 0:1], rs_row[:, mj * 128:(mj + 1) * 128],
                            onesf[0:1, 0:1])
                        nc.scalar.copy(invrowL[:, t * 2 + mj: t * 2 + mj + 1],
                                       irT[:, 0:1])

                # aggr sums + deg
                agg_ps = ps.tile([128, 16], F32, tag="agg")
                for kt in range(NT):
                    for mj in range(2):
                        nc.tensor.matmul(
                            agg_ps[:, mj * 8:(mj + 1) * 8],
                            cb[:, kt * P + mj * 128: kt * P + mj * 128 + 128],
                            xe[:, kt * 8: kt * 8 + 8],
                            start=(kt == 0 and mj == 0),
                            stop=(kt == NT - 1 and mj == 1))

                xi_ps = ps.tile([128, 256], F32, tag="xi")
                amT = sb.tile([8, 256], BF16, tag="amTs")
                for mj in range(2):
                    deg = sb.tile([128, 1], F32, tag="deg")
                    nc.vector.tensor_scalar_max(
                        deg[:, :], agg_ps[:, mj * 8 + 6: mj * 8 + 7], 1.0)
                    invdeg = sb.tile([128, 1], F32, tag="invdeg")
                    nc.vector.reciprocal(invdeg[:, :], deg[:, :])
                    am = sb.tile([128, 8], BF16, tag="am")
                    nc.vector.tensor_scalar_mul(
                        am[:, :], agg_ps[:, mj * 8:(mj + 1) * 8], invdeg[:, :])
                    amT_ps = ps.tile([8, 128], BF16, tag="tr")
                    nc.tensor.transpose(amT_ps[:, :], am[:, :], ident[:, :])
                    nc.scalar.copy(amT[:, mj * 128:(mj + 1) * 128],
                                   amT_ps[:, :])

                for mj in range(2):
                    sl = slice(mj * 128, (mj + 1) * 128)
                    nc.tensor.matmul(xi_ps[:, sl], amT[0:6, sl],
                                     wlT_sb[0:6, :], start=(mj == 0),
                                     stop=False)
                    nc.tensor.matmul(xi_ps[:, sl], xT_sb[0:6, sl],
                                     wrT_sb[0:6, :], start=False, stop=False)
                    nc.tensor.matmul(xi_ps[:, sl], ones1[:, :], bl_sb[:, :],
                                     start=False, stop=(mj == 1))

                for mj in range(2):
                    sl = slice(mj * 128, (mj + 1) * 128)
                    ssq = sb.tile([128, 1], F32, tag="ssq")
                    sqscr = sb.tile([128, H], F32, tag="sqscr")
                    nc.scalar.activation(sqscr[:, :], xi_ps[:, sl], AF.Square,
                                         accum_out=ssq[:, :])
                    nrm = sb.tile([128, 1], F32, tag="nrm")
                    nc.scalar.sqrt(nrm[:, :], ssq[:, :])
                    nc.vector.tensor_scalar_max(nrm[:, :], nrm[:, :], 1e-12)
                    invn = sb.tile([128, 1], F32, tag="invn")
                    nc.vector.reciprocal(invn[:, :], nrm[:, :])
                    # unscaled xi -> xiT (map-LSTM input)
                    xin = sb.tile([128, H], BF16, tag="xin")
                    nc.scalar.activation(xin[:, :], xi_ps[:, sl], AF.Relu,
                                         scale=invn[:, :])
                    trp = ps.tile([128, 128], BF16, tag="tr")
                    nc.tensor.transpose(trp[:, :], xin[:, :], ident[:, :])
                    nc.scalar.copy(
                        xiT[:, t * P + mj * 128: t * P + mj * 128 + 128],
                        trp[:, :])
                    if t < 13:
                        # pre-scaled by invrow_{t+1} (own rows) for the gather
                        invne = sb.tile([128, 1], F32, tag="invne")
                        nc.vector.tensor_mul(
                            invne[:, :], invn[:, :],
                            invrowL[:, t * 2 + mj: t * 2 + mj + 1])
                        xins = sb.tile([128, H], BF16, tag="xins")
                        nc.scalar.activation(xins[:, :], xi_ps[:, sl], AF.Relu,
                                             scale=invne[:, :])
                        nc.sync.dma_start(
                            out=xi_sh[t * P + mj * 128:
                                      t * P + mj * 128 + 128, :],
                            in_=xins[:, :])

            # ---------- collective: xi (2 chunks, overlap phase 1) ----------
            nc.gpsimd.collective_compute(
                "AllGather", mybir.AluOpType.bypass, replica_groups=RG,
                ins=[xi_sh[0:6 * P, :].opt()], outs=[xi_gA.ap().opt()])
            nc.gpsimd.collective_compute(
                "AllGather", mybir.AluOpType.bypass, replica_groups=RG,
                ins=[xi_sh[6 * P:13 * P, :].opt()], outs=[xi_gB.ap().opt()])

            # ---------- phase 2: merged1_t = A_norm_t @ xi_{t-1} ----------
            for t in range(1, T):
                xg = sb3.tile([128, NT * H], BF16, tag="xg")
                for r in range(NCORES):
                    if t - 1 < 6:
                        row = (r * 6 + (t - 1)) * P
                        src = xi_gA[row: row + P, :]
                    else:
                        row = (r * 7 + (t - 1 - 6)) * P
                        src = xi_gB[row: row + P, :]
                    nc.sync.dma_start(
                        out=xg[:, r * 2 * H:(r + 1) * 2 * H]
                        .rearrange("p (q h) -> p q h", h=H),
                        in_=src.rearrange("(q p) h -> p q h", p=128))
                m1_ps = ps.tile([128, P], F32, tag="mg")
                for kt in range(NT):
                    nc.tensor.matmul(
                        m1_ps[:, :], xg[:, kt * H:(kt + 1) * H],
                        tres[:, ((t - 1) * NT + kt) * P:
                             ((t - 1) * NT + kt + 1) * P],
                        start=(kt == 0), stop=(kt == NT - 1))
                nc.scalar.copy(m1T[:, t * P:(t + 1) * P], m1_ps[:, :])
                if t <= 12:
                    # transpose to natural [node, h], then scale rows by
                    # invrow_{t+1} (node on partitions) for the gather
                    m1bf = sb.tile([128, P], BF16, tag="m1bf")
                    nc.vector.tensor_copy(m1bf[:, :], m1_ps[:, :])
                    for mj in range(2):
                        trp = ps.tile([128, 128], BF16, tag="tr")
                        nc.tensor.transpose(
                            trp[:, :], m1bf[:, mj * 128:(mj + 1) * 128],
                            ident[:, :])
                        nnat = sb.tile([128, 128], BF16, tag="nnat")
                        nc.vector.tensor_scalar_mul(
                            nnat[:, :], trp[:, :],
                            invrowL[:, t * 2 + mj: t * 2 + mj + 1])
                        nc.sync.dma_start(
                            out=m1_sh[(t - 1) * P + mj * 128:
                                      (t - 1) * P + mj * 128 + 128, :],
                            in_=nnat[:, :])

            nc.gpsimd.collective_compute(
                "AllGather", mybir.AluOpType.bypass, replica_groups=RG,
                ins=[m1_sh[0:6 * P, :].opt()], outs=[m1_gA.ap().opt()])
            nc.gpsimd.collective_compute(
                "AllGather", mybir.AluOpType.bypass, replica_groups=RG,
                ins=[m1_sh[6 * P:12 * P, :].opt()], outs=[m1_gB.ap().opt()])

            # ---------- phase 3: merged0_t = A_norm_t @ merged1_{t-1} ----------
            for t in range(2, T):
                mg_in = sb3.tile([128, NT * H], BF16, tag="xg")
                for r in range(NCORES):
                    if t - 2 < 6:
                        row = (r * 6 + (t - 2)) * P
                        src = m1_gA[row: row + P, :]
                    else:
                        row = (r * 6 + (t - 2 - 6)) * P
                        src = m1_gB[row: row + P, :]
                    nc.sync.dma_start(
                        out=mg_in[:, r * 2 * H:(r + 1) * 2 * H]
                        .rearrange("p (q h) -> p q h", h=H),
                        in_=src.rearrange("(q p) h -> p q h", p=128))
                m0_ps = ps.tile([128, P], F32, tag="mg")
                for kt in range(NT):
                    nc.tensor.matmul(
                        m0_ps[:, :], mg_in[:, kt * H:(kt + 1) * H],
                        tres[:, ((t - 1) * NT + kt) * P:
                             ((t - 1) * NT + kt + 1) * P],
                        start=(kt == 0), stop=(kt == NT - 1))
                nc.scalar.copy(m0T[:, t * P:(t + 1) * P], m0_ps[:, :])

            # ---------- phase 4: map LSTM over batch 3584 ----------
            xsides = [m0T, m1T, xiT]
            for b in range(NCH):
                sl = slice(b * CH, (b + 1) * CH)
                for k in range(KSEQ):
                    gate = {}
                    for g in range(4):
                        if k == 0 and g == 1:
                            continue
                        z_ps = ps2.tile([128, CH], F32, tag="z")
                        nc.tensor.matmul(z_ps[:, :],
                                         mwihT_bf[:, g * H:(g + 1) * H],
                                         xsides[k][:, sl],
                                         start=True, stop=(k == 0))
                        if k > 0:
                            nc.tensor.matmul(z_ps[:, :],
                                             mwhhT_r[:, g * H:(g + 1) * H],
                                             hT[:, sl], start=False, stop=True)
                        func = AF.Tanh if g == 2 else AF.Sigmoid
                        gt = sb1.tile([128, CH], F32, tag=f"gate{g}")
                        nc.scalar.activation(gt[:, :], z_ps[:, :], func,
                                             bias=mb_sb[:, g:g + 1])
                        gate[g] = gt
                    if k == 0:
                        nc.vector.tensor_mul(cT[:, sl], gate[0][:, :],
                                             gate[2][:, :])
                    else:
                        tmp = sb1.tile([128, CH], F32, tag="tmp")
                        nc.vector.tensor_mul(tmp[:, :], gate[0][:, :],
                                             gate[2][:, :])
                        nc.vector.tensor_mul(cT[:, sl], gate[1][:, :],
                                             cT[:, sl])
                        nc.vector.tensor_add(cT[:, sl], cT[:, sl], tmp[:, :])
                    tanc = sb1.tile([128, CH], F32, tag="tanc")
                    nc.scalar.activation(tanc[:, :], cT[:, sl], AF.Tanh)
                    nc.vector.tensor_mul(hT[:, sl], gate[3][:, :], tanc[:, :])

            # ---------- phase 5: temporal LSTM (12 steps, batch 256) ----------
            h2_top = big.tile([128, P], F32R, tag="h2top")
            h2_tail = big.tile([8, P], F32R, tag="h2tail")
            c2_top = big.tile([128, P], F32, tag="c2top")
            c2_tail = big.tile([8, P], F32, tag="c2tail")
            for ti in range(LEN):
                t = ti + 2
                og = sb1.tile([8, P], F32, tag="og")
                nc.sync.dma_start(out=og[:, :], in_=obsgamT[t])
                og_r = sb1.tile([8, P], F32R, tag="ogr")
                nc.scalar.copy(og_r[:, :], og[:, :])
                xtop = hT[:, t * P:(t + 1) * P]

                z_main = []
                for g in range(4):
                    if ti == 0 and g == 1:
                        z_main.append(None)
                        continue
                    z_ps = ps2.tile([128, P], F32, tag="z")
                    nc.tensor.matmul(z_ps[:, :],
                                     lwihT_top_r[:, g * H:(g + 1) * H],
                                     xtop, start=True, stop=False)
                    nc.tensor.matmul(z_ps[:, :],
                                     lwihT_tail_r[:, g * H:(g + 1) * H],
                                     og_r[:, :], start=False, stop=(ti == 0))
                    if ti > 0:
                        nc.tensor.matmul(z_ps[:, :],
                                         lwhhT_top_r[:, g * H:(g + 1) * H],
                                         h2_top[:, :], start=False, stop=False)
                        nc.tensor.matmul(z_ps[:, :],
                                         lwhhT_tail_r[:, g * H:(g + 1) * H],
                                         h2_tail[:, :], start=False, stop=True)
                    z_main.append(z_ps)
                # tail gates padded to partitions 32g..32g+8 of one psum tile
                zt_ps = ps.tile([128, P], F32, tag="ztail")
                nc.tensor.matmul(zt_ps[:, :], lwihT_top_r[:, 512:640], xtop,
                                 start=True, stop=False)
                nc.tensor.matmul(zt_ps[:, :], lwihT_tail_r[:, 512:640],
                                 og_r[:, :], start=False, stop=(ti == 0))
                if ti > 0:
                    nc.tensor.matmul(zt_ps[:, :], lwhhT_top_r[:, 512:640],
                                     h2_top[:, :], start=False, stop=False)
                    nc.tensor.matmul(zt_ps[:, :], lwhhT_tail_r[:, 512:640],
                                     h2_tail[:, :], start=False, stop=True)

                gates_m, gates_t = {}, {}
                for g in range(4):
                    if z_main[g] is None:
                        continue
                    func = AF.Tanh if g == 2 else AF.Sigmoid
                    gm = sb1.tile([128, P], F32, tag=f"tg{g}")
                    nc.scalar.activation(gm[:, :], z_main[g][:, :], func,
                                         bias=lbm_sb[:, g:g + 1])
                    gates_m[g] = gm
                    gtl = sb1.tile([8, P], F32, tag=f"tt{g}")
                    nc.scalar.activation(gtl[:, :],
                                         zt_ps[32 * g:32 * g + 8, :], func,
                                         bias=lbt_sb[32 * g:32 * g + 8, :])
                    gates_t[g] = gtl
                for part, gm, c_, h_ in ((128, gates_m, c2_top, h2_top),
                                         (8, gates_t, c2_tail, h2_tail)):
                    if 1 in gm:
                        tmp = sb1.tile([part, P], F32, tag=f"ttmp{part}")
                        nc.vector.tensor_mul(tmp[:, :], gm[0][:, :],
                                             gm[2][:, :])
                        nc.vector.tensor_mul(c_[:, :], gm[1][:, :], c_[:, :])
                        nc.vector.tensor_add(c_[:, :], c_[:, :], tmp[:, :])
                    else:
                        nc.vector.tensor_mul(c_[:, :], gm[0][:, :],
                                             gm[2][:, :])
                    tct = sb1.tile([part, P], F32, tag=f"ttanc{part}")
                    nc.scalar.activation(tct[:, :], c_[:, :], AF.Tanh)
                    nc.vector.tensor_mul(h_[:, :], gm[3][:, :], tct[:, :])

            # ---------- phase 6: head ----------
            h1_ps = ps2.tile([128, P], F32, tag="z")
            nc.tensor.matmul(h1_ps[:, :], linwT_top_r[:, 0:128], h2_top[:, :],
                             start=True, stop=False)
            nc.tensor.matmul(h1_ps[:, :], linwT_tail_r[:, 0:128],
                             h2_tail[:, :], start=False, stop=True)
            h1t_ps = ps.tile([128, P], F32, tag="ztail")
            nc.tensor.matmul(h1t_ps[0:8, :], linwT_top_r[:, 128:136],
                             h2_top[:, :], start=True, stop=False)
            nc.tensor.matmul(h1t_ps[0:8, :], linwT_tail_r[:, 128:136],
                             h2_tail[:, :], start=False, stop=True)
            h1_top = sb1.tile([128, P], F32R, tag="h1top")
            nc.scalar.activation(h1_top[:, :], h1_ps[:, :], AF.Relu,
                                 bias=linbm_sb[:, :])
            h1_tail = sb1.tile([8, P], F32R, tag="h1tail")
            nc.scalar.activation(h1_tail[:, :], h1t_ps[0:8, :], AF.Relu,
                                 bias=linbt_sb[:, :])
            o_ps = ps2.tile([128, P], F32, tag="z")
            nc.tensor.matmul(o_ps[0:2, :], lin1wT_top_r[:, :], h1_top[:, :],
                             start=True, stop=False)
            nc.tensor.matmul(o_ps[0:2, :], lin1wT_tail_r[:, :], h1_tail[:, :],
                             start=False, stop=True)
            o_sb = sb1.tile([2, P], F32, tag="osb")
            nc.scalar.activation(o_sb[:, :], o_ps[0:2, :], AF.Identity,
                                 bias=lin1b_sb[:, :])
            nc.sync.dma_start(out=out_ext[:, :], in_=o_sb[:, :])

    nc._dbg_names = {
        "m0T": m0T.tensor.name, "m1T": m1T.tensor.name,
        "xiT": xiT.tensor.name, "hT": hT.tensor.name,
        "h2_top": h2_top.tensor.name, "h2_tail": h2_tail.tensor.name,
    }
    nc.compile()
    return nc


def prep_inputs(inputs):
    a = np.asarray(inputs["a_queue"])
    x = np.asarray(inputs["x_queue"], np.float32)
    obs = np.asarray(inputs["obs_queue"], np.float32)
    gam = np.asarray(inputs["u_gamma_queue"], np.float32)

    a8 = a.astype(ml_dtypes.float8_e4m3)
    xext = np.zeros((T, N, 8), ml_dtypes.bfloat16)
    xext[:, :, :6] = x.astype(ml_dtypes.bfloat16)
    xext[:, :, 6] = 1.0

    perm = []
    for g in range(4):
        perm.extend(range(g * D, g * D + 128))
    for g in range(4):
        perm.extend(range(g * D + 128, (g + 1) * D))

    def gate_perm_pad(w):
        # [544, 136] -> permuted-transposed, tails padded to 32-offsets
        wp = np.asarray(w, np.float32)[perm].T  # [136, 544]
        out = np.zeros((136, 640), np.float32)
        out[:, :512] = wp[:, :512]
        for g in range(4):
            out[:, 512 + 32 * g: 512 + 32 * g + 8] = \
                wp[:, 512 + 8 * g: 512 + 8 * g + 8]
        return np.ascontiguousarray(out)

    lwihT = gate_perm_pad(inputs["lstm_wih"])
    lwhhT = gate_perm_pad(inputs["lstm_whh"])
    lb = (np.asarray(inputs["lstm_bih"], np.float32)
          + np.asarray(inputs["lstm_bhh"], np.float32))[perm]
    lbt = np.zeros((128, 1), np.float32)
    for g in range(4):
        lbt[32 * g:32 * g + 8, 0] = lb[512 + 8 * g: 512 + 8 * g + 8]
    mb = (np.asarray(inputs["map_bih"], np.float32)
          + np.asarray(inputs["map_bhh"], np.float32))

    linwT = np.ascontiguousarray(np.asarray(inputs["lin_w"], np.float32).T)
    lin1wT = np.ascontiguousarray(np.asarray(inputs["lin1_w"], np.float32).T)

    shared = {
        "xext": xext,
        "identin": np.eye(128, dtype=ml_dtypes.bfloat16),
        "wlT": np.pad(np.asarray(inputs["sage_wl"], np.float32).T,
                      ((0, 2), (0, 0))).astype(ml_dtypes.bfloat16),
        "wrT": np.pad(np.asarray(inputs["sage_wr"], np.float32).T,
                      ((0, 2), (0, 0))).astype(ml_dtypes.bfloat16),
        "blrow": np.asarray(inputs["sage_bl"], np.float32)[None, :]
        .astype(ml_dtypes.bfloat16),
        "mwihT": np.ascontiguousarray(
            np.asarray(inputs["map_wih"], np.float32).T),
        "mwhhT": np.ascontiguousarray(
            np.asarray(inputs["map_whh"], np.float32).T),
        "mbias": np.ascontiguousarray(mb.reshape(4, 128).T),
        "lwihT_top": np.ascontiguousarray(lwihT[:128]),
        "lwihT_tail": np.ascontiguousarray(lwihT[128:]),
        "lwhhT_top": np.ascontiguousarray(lwhhT[:128]),
        "lwhhT_tail": np.ascontiguousarray(lwhhT[128:]),
        "lb_main": np.ascontiguousarray(lb[:512].reshape(4, 128).T),
        "lb_tail": lbt,
        "linwT_top": np.ascontiguousarray(linwT[:128]),
        "linwT_tail": np.ascontiguousarray(linwT[128:]),
        "linb_main": np.asarray(inputs["lin_b"], np.float32)[:128, None],
        "linb_tail": np.asarray(inputs["lin_b"], np.float32)[128:, None],
        "lin1wT_top": np.ascontiguousarray(lin1wT[:128]),
        "lin1wT_tail": np.ascontiguousarray(lin1wT[128:]),
        "lin1b": np.asarray(inputs["lin1_b"], np.float32)[:, None],
    }

    in_maps = []
    for c in range(NCORES):
        blk = slice(c * P, (c + 1) * P)
        m = dict(shared)
        m["tblk"] = np.ascontiguousarray(a8[1:, blk, :].transpose(0, 2, 1))
        m["cblk"] = np.ascontiguousarray(a8[:, :, blk])
        m["xT"] = np.ascontiguousarray(
            np.pad(x[:, blk, :], ((0, 0), (0, 0), (0, 2)))
            .transpose(0, 2, 1).astype(ml_dtypes.bfloat16))
        ogT = np.concatenate([obs[:, blk, :], gam[:, blk, :]], axis=2)
        m["obsgamT"] = np.ascontiguousarray(ogT.transpose(0, 2, 1))
        in_maps.append(m)
    return in_maps


def kernel(**inputs):
    if "nc" not in _cached:
        _cached["nc"] = build()
    nc = _cached["nc"]
    in_maps = prep_inputs(inputs)
    results = bass2jax.run_bass_via_pjrt(nc, in_maps, NCORES)
    out = np.concatenate([np.asarray(r["out"]).T for r in results], axis=0)
    return out.astype(np.float32)


if __name__ == "__main__":
    build()
    print("build ok")

